# revision 2
# baseline (speedup 1.0000x reference)
"""FJSP decoder kernel for Trainium2, data-parallel over batch on 8 NeuronCores.

Same factorized-softmax math as the baseline:
  q/k/v of pair s=(j,m) decompose as x[s]=xj[j]+xm[m]; the joint softmax
  factorizes exactly and Wmhc @ Wshc collapses to per-head scalars uv,
  so the decoder reduces to [100,20]-shaped per-head work.

v3 restructuring for latency:
  - ALL layout marshalling on host: ej/em pre-transposed, weights pre-padded
    into 32-strip head layout, w2/bias folded -> two bf16 input DMAs
    (q/k path first, v/w2/mask second).
  - bf16 matmul inputs everywhere (1 PE cycle/col vs 4 for fp32); an early
    dummy matmul starts the PE p-state ramp during the DMA.
  - machine-key score blocks for 4 heads stacked on partition strips via
    tile_position -> one exp instruction per head-group covers A|C|B|D.
  - PSUM split into per-purpose bank-granular tiles so tile-level dependency
    tracking doesn't serialize unrelated phases.
  - uv scalings on GPSIMD, combine per group overlapped with the other
    group's exp; tanh directly; short final chain.
"""

import math

import numpy as np
import ml_dtypes

import concourse.bass as bass
import concourse.mybir as mybir
import concourse.tile as tile
from concourse.bass_utils import run_bass_kernel_spmd

F32 = mybir.dt.float32
BF16 = mybir.dt.bfloat16
AF = mybir.ActivationFunctionType
OP = mybir.AluOpType
AX = mybir.AxisListType

D, H, QD = 128, 8, 16
B, J, M = 8, 100, 20
INV_SQ = 1.0 / math.sqrt(QD)
SD = math.sqrt(D)

# data1 (bf16): ejT | emT | 8 q/k weight tiles | bias
EJ0, EM0, WT1 = 0, 100, 120
BIAS0 = WT1 + 8 * 128           # 1144
D1W = BIAS0 + 1                 # 1145
# data2 (bf16): 4 v weight tiles | w2pad G0,G1 | mask
WT2 = 0
W20 = WT2 + 4 * 128             # 512
MK0 = W20 + 2                   # 514
D2W = MK0 + M                   # 534


def _woff1(G, nm, half):        # nm: 0=q 1=k
    return WT1 + ((G * 2 + nm) * 2 + half) * 128


def _woff2(G, half):            # v tiles
    return WT2 + (G * 2 + half) * 128


# e1 (sbuf, bf16) column layout per group:
# 0:480 A^T|C^T (head t at 120t), 480:512 junk, 512:612 B^T stack,
# 612:632 D^T stack, 632:712 eCu (head t at 632+20t), 712:732 eDu
E1W = 732

_PATCHED = False


def _install_drain_patch():
    global _PATCHED
    if _PATCHED:
        return
    from concourse.tile import ScopedClock, TileContext

    def _split_drain_and_barrier(self, tick_clock, wait_clock):
        drain_inst = self.nc.sync.drain()
        wait_clock.add_sem_waits(
            drain_inst.ins, ScopedClock({None: tick_clock.global_clock})
        )
        si = drain_inst.ins.sync_info
        waits = list(si.on_wait) if si is not None else []
        if len(waits) > 1:
            assert not si.on_update
            sems = {s.name: s for s in self.sems.allocated().values()}
            drain_inst.ins.sync_info = None
            drain_inst.wait_op(sems[waits[0].ant_name], waits[0].wait_value, "sem-ge")
            engines = [
                self.nc.scalar,
                self.nc.vector,
                self.nc.tensor,
                self.nc.gpsimd,
                self.nc.sync,
            ]
            for i, w in enumerate(waits[1:]):
                extra = engines[i % len(engines)].drain()
                extra.wait_op(sems[w.ant_name], w.wait_value, "sem-ge")
        self.nc.all_engine_barrier()
        assert self.sems is not None
        popped = self.nc._tile_sem_poison_stack.pop()
        assert popped is self._sem_poison
        self.nc.clear_and_free_semaphores(list(self.sems.allocated().values()))

    TileContext._drain_and_barrier = _split_drain_and_barrier
    _PATCHED = True


def _split_multi_waits(nc):
    import bass_rust

    ctr = 0
    for fn in nc.m.functions:
        for bb in fn.blocks:
            il = bb.instructions
            if not any(
                i.sync_info is not None and len(i.sync_info.on_wait) > 1 for i in il
            ):
                continue
            new = []
            for ins in il:
                si = ins.sync_info
                if si is not None and len(si.on_wait) > 1:
                    waits = list(si.on_wait)
                    ups = list(si.on_update)
                    for w in waits[:-1]:
                        nop = mybir.InstNoOp(name=f"I-waitsplit-{ctr}", ins=[], outs=[])
                        ctr += 1
                        nop.engine = ins.engine
                        nop.sync_info = bass_rust.SyncInfo(on_update=[], on_wait=[w])
                        new.append(nop)
                    ins.sync_info = bass_rust.SyncInfo(
                        on_update=ups, on_wait=[waits[-1]]
                    )
                new.append(ins)
            bb.instructions = new


def _insert(ap_slice, dim_pair, at=1):
    ap = list(ap_slice.ap)
    ap2 = ap[:at] + [list(dim_pair)] + ap[at:]
    return bass.AP(tensor=ap_slice.tensor, offset=ap_slice.offset, ap=ap2)


def _append(ap_slice, dim_pair):
    ap = list(ap_slice.ap) + [list(dim_pair)]
    return bass.AP(tensor=ap_slice.tensor, offset=ap_slice.offset, ap=ap)


def _chunk2(ap_slice, chunk_step):
    return _insert(ap_slice, [chunk_step, 2], at=1)


def _build():
    nc = bass.Bass()
    d1_d = nc.dram_tensor("data1", [D, D1W], BF16, kind="ExternalInput")
    d2_d = nc.dram_tensor("data2", [D, D2W], BF16, kind="ExternalInput")
    out_d = nc.dram_tensor("out", [J, M], F32, kind="ExternalOutput")

    with tile.TileContext(nc) as tc:
        with (
            tc.tile_pool(name="persist", bufs=1) as pp,
            tc.tile_pool(name="ps", bufs=1, space="PSUM") as psp,
        ):
            # PSUM: 8 banks exactly
            ps_pj0 = psp.tile([D, 512], F32, tag="pj0")   # q0|k0 proj + uv + tot + warm
            ps_pj1 = psp.tile([D, 512], F32, tag="pj1")   # q1|k1|v0|v1 proj
            ps_sc0 = psp.tile([D, 1024], F32, tag="sc0")  # G0 scores: A 0:480, BD 512:632
            ps_sc1 = psp.tile([D, 1024], F32, tag="sc1")
            ps_sn0 = psp.tile([D, 512], F32, tag="sn0")   # G0 [SE|Nj|SF|Nm] x4
            ps_sn1 = psp.tile([D, 512], F32, tag="sn1")

            # ---- warmup + constants (run while DMA is in flight) ---------
            dum = pp.tile([D, 4], BF16, tag="dum")
            nc.vector.memset(dum[0:1, :], 0.5)
            nc.tensor.matmul(out=ps_pj0[0:1, 400:401],
                             lhsT=dum[0:1, 0:1], rhs=dum[0:1, 1:2])
            ones = pp.tile([D, J], F32, tag="ones")
            nc.gpsimd.memset(ones, 1.0)

            # ---- input DMAs ----------------------------------------------
            data1 = pp.tile([D, D1W], BF16, tag="data1")
            nc.sync.dma_start(out=data1, in_=d1_d[:])
            data2 = pp.tile([D, D2W], BF16, tag="data2")
            nc.sync.dma_start(out=data2, in_=d2_d[:])

            ejT = data1[:, EJ0 : EJ0 + J]
            emT = data1[:, EM0 : EM0 + M]
            biasc = data1[0:J, BIAS0 : BIAS0 + 1]
            maskc = data2[0:J, MK0 : MK0 + M]

            # ---- projections (PE, bf16) ----------------------------------
            # ps_pj0: q G0 at 0:120, k G0 at 128:248
            # ps_pj1: q G1 0:120, k G1 128:248, v G0 256:376, v G1 384:504
            def proj(ps, base, dat, woff):
                nc.tensor.matmul(out=ps[:, base : base + J],
                                 lhsT=dat[:, woff(0) : woff(0) + 128], rhs=ejT)
                nc.tensor.matmul(out=ps[:, base + J : base + J + M],
                                 lhsT=dat[:, woff(1) : woff(1) + 128], rhs=emT)

            proj(ps_pj0, 0, data1, lambda h: _woff1(0, 0, h))    # q G0
            proj(ps_pj0, 128, data1, lambda h: _woff1(0, 1, h))  # k G0
            proj(ps_pj1, 0, data1, lambda h: _woff1(1, 0, h))    # q G1
            proj(ps_pj1, 128, data1, lambda h: _woff1(1, 1, h))  # k G1
            proj(ps_pj1, 256, data2, lambda h: _woff2(0, h))     # v G0
            proj(ps_pj1, 384, data2, lambda h: _woff2(1, h))     # v G1

            qt = [pp.tile([D, 120], BF16, tag=f"qt{G}", name=f"qt{G}") for G in range(2)]
            kt = [pp.tile([D, 120], BF16, tag=f"kt{G}", name=f"kt{G}") for G in range(2)]
            vt = [pp.tile([D, 120], BF16, tag=f"vt{G}", name=f"vt{G}") for G in range(2)]
            nc.vector.tensor_copy(out=qt[0], in_=ps_pj0[:, 0:120])
            nc.scalar.copy(out=kt[0], in_=ps_pj0[:, 128:248])
            nc.vector.tensor_copy(out=qt[1], in_=ps_pj1[:, 0:120])
            nc.scalar.copy(out=kt[1], in_=ps_pj1[:, 128:248])
            nc.vector.tensor_copy(out=vt[0], in_=ps_pj1[:, 256:376])
            nc.vector.tensor_copy(out=vt[1], in_=ps_pj1[:, 384:504])

            # ---- score + uv matmuls (PE) ---------------------------------
            ps_sc = (ps_sc0, ps_sc1)
            e1 = [pp.tile([D, E1W], BF16, tag=f"e1g{G}", name=f"e1g{G}") for G in range(2)]
            for G in range(2):
                q, k = qt[G], kt[G]
                for t in range(4):
                    s = slice(32 * t, 32 * t + 32)
                    nc.tensor.matmul(  # stacked B^T|D^T
                        out=ps_sc[G][32 * t : 32 * t + 20, 512:632],
                        lhsT=k[s, 100:120],
                        rhs=q[s, 0:120],
                        tile_position=(32 * t, 32 * t),
                    )
                    nc.tensor.matmul(  # A^T|C^T head 4G+t
                        out=ps_sc[G][0:J, 120 * t : 120 * t + 120],
                        lhsT=k[s, 0:100],
                        rhs=q[s, 0:120],
                        tile_position=(32 * t, 0),
                    )
            for G in range(2):
                w2c = data2[:, W20 + G : W20 + G + 1]
                for t in range(4):
                    s = slice(32 * t, 32 * t + 32)
                    nc.tensor.matmul(  # uvj head 4G+t -> ps_pj0 col 256+4G+t
                        out=ps_pj0[0:J, 256 + 4 * G + t : 257 + 4 * G + t],
                        lhsT=vt[G][s, 0:100],
                        rhs=w2c[s, :],
                        tile_position=(32 * t, 0),
                    )
                    nc.tensor.matmul(  # uvm stacked -> ps_pj0 col 264+G
                        out=ps_pj0[32 * t : 32 * t + 20, 264 + G : 265 + G],
                        lhsT=vt[G][s, 100:120],
                        rhs=w2c[s, :],
                        tile_position=(32 * t, 32 * t),
                    )

            # ---- exps (Act): one per group -------------------------------
            for G in range(2):
                nc.scalar.activation(
                    out=e1[G][0:116, 0:632],
                    in_=ps_sc[G][0:116, 0:632],
                    func=AF.Exp,
                    scale=INV_SQ,
                )

            # ---- uv copy (DVE) + scalings (Pool) -------------------------
            uv_sb = pp.tile([D, 10], F32, tag="uv")
            nc.vector.tensor_copy(out=uv_sb[0:116, :], in_=ps_pj0[0:116, 256:266])
            for G in range(2):
                in0 = _insert(e1[G][0:J, 100:120], [120, 4])
                in1 = _append(uv_sb[0:J, 4 * G : 4 * G + 4], [0, 20])
                outv = _insert(e1[G][0:J, 632:652], [20, 4])
                nc.gpsimd.tensor_mul(out=outv, in0=in0, in1=in1)
                nc.gpsimd.tensor_mul(
                    out=e1[G][0:116, 712:732],
                    in0=e1[G][0:116, 612:632],
                    in1=_append(uv_sb[0:116, 8 + G : 9 + G], [0, 20]),
                )

            # ---- [SE|Nj] and [SF|Nm] matmuls (PE) ------------------------
            ps_sn = (ps_sn0, ps_sn1)
            for G in range(2):
                for t in range(4):
                    sn = 80 * t
                    nc.tensor.matmul(  # [SE|Nj]
                        out=ps_sn[G][0:J, sn : sn + 40],
                        lhsT=e1[G][0:J, 120 * t : 120 * t + 100],
                        rhs=_chunk2(
                            e1[G][0:J, 120 * t + 100 : 120 * t + 120], 532 - 100 * t
                        ),
                    )
                    nc.tensor.matmul(  # [SF|Nm]
                        out=ps_sn[G][0:J, sn + 40 : sn + 80],
                        lhsT=e1[G][32 * t : 32 * t + 20, 512:612],
                        rhs=_chunk2(e1[G][32 * t : 32 * t + 20, 612:632], 100),
                        tile_position=(32 * t, 0),
                    )

            # ---- combine (DVE): c8 = N / S, reduce over heads ------------
            rs = pp.tile([J, 320], F32, tag="rs")
            c8 = pp.tile([J, 320], F32, tag="c8")
            for G in range(2):
                sview = _insert(ps_sn[G][0:J, 0:20], [40, 8])
                nview = _insert(ps_sn[G][0:J, 20:40], [40, 8])
                rsv = _insert(rs[0:J, 160 * G : 160 * G + 20], [20, 8])
                c8v = _insert(c8[0:J, 160 * G : 160 * G + 20], [20, 8])
                nc.vector.reciprocal(out=rsv, in_=sview)
                nc.vector.tensor_mul(out=c8v, in0=nview, in1=rsv)
            c1 = pp.tile([J, M], F32, tag="c1")
            red_in = bass.AP(
                tensor=c8[:].tensor,
                offset=c8[0:J, 0:M].offset,
                ap=[c8[0:J, 0:M].ap[0], [1, 20], [20, 16]],
            )
            nc.vector.tensor_reduce(out=c1, in_=red_in, axis=AX.X, op=OP.add)

            # ---- final chain ---------------------------------------------
            t_sb = pp.tile([J, M], F32, tag="t")
            nc.scalar.activation(
                out=t_sb, in_=c1, func=AF.Tanh, scale=1.0 / SD, bias=biasc
            )
            arg = pp.tile([J, M], F32, tag="arg")
            nc.vector.scalar_tensor_tensor(
                out=arg, in0=t_sb, scalar=10.0, in1=maskc, op0=OP.mult, op1=OP.add
            )
            e_sb = pp.tile([J, M], F32, tag="e")
            s_row = pp.tile([J, 1], F32, tag="srow")
            nc.scalar.activation(
                out=e_sb, in_=arg, func=AF.Exp, scale=1.0, accum_out=s_row
            )
            nc.tensor.matmul(
                out=ps_pj0[0:J, 384:385], lhsT=ones[0:J, 0:J], rhs=s_row
            )
            rtot = pp.tile([J, 1], F32, tag="rtot")
            nc.vector.reciprocal(out=rtot, in_=ps_pj0[0:J, 384:385])
            out_t = pp.tile([J, M], F32, tag="outt")
            nc.vector.tensor_scalar_mul(out=out_t, in0=e_sb, scalar1=rtot)
            nc.sync.dma_start(out=out_d[:], in_=out_t)

    _split_multi_waits(nc)
    return nc


_NC = None
last_results = None


def _pack_static(inputs):
    """Weight columns of data1/data2 (identical across batches)."""
    w2 = (np.asarray(inputs["Wmhc"], np.float64)
          @ np.asarray(inputs["Wshc"], np.float64))[:, 0]
    bias0 = float(
        (np.asarray(inputs["b_mhc"], np.float64)
         @ np.asarray(inputs["Wshc"], np.float64)
         + np.asarray(inputs["b_shc"], np.float64)).reshape(-1)[0]
    )
    d1 = np.zeros((D, D1W), np.float32)
    d2 = np.zeros((D, D2W), np.float32)
    Wq = np.asarray(inputs["Wq3"], np.float32)
    Wk = np.asarray(inputs["Wk"], np.float32)
    Wv = np.asarray(inputs["Wv"], np.float32)

    def fill(dst, off, Wh, G):
        for s in range(4):
            dst[:, off + 32 * s : off + 32 * s + 16] = Wh[
                :, 64 * G + 16 * s : 64 * G + 16 * s + 16
            ]

    for G in range(2):
        for half in range(2):
            fill(d1, _woff1(G, 0, half), (Wq[:D] if half == 0 else Wq[D:]), G)
            fill(d1, _woff1(G, 1, half), (Wk[:D] if half == 0 else Wk[D:]), G)
            fill(d2, _woff2(G, half), (Wv[:D] if half == 0 else Wv[D:]), G)
        for s in range(4):
            d2[32 * s : 32 * s + 16, W20 + G] = w2[
                64 * G + 16 * s : 64 * G + 16 * s + 16
            ]
    d1[0:J, BIAS0] = bias0 / SD
    return d1, d2


def kernel(**inputs):
    global _NC, last_results
    _install_drain_patch()
    if _NC is None:
        _NC = _build()

    d1s, d2s = _pack_static(inputs)
    ejs = np.asarray(inputs["encoded_job"], np.float32)
    ems = np.asarray(inputs["encoded_machine"], np.float32)
    msks = np.asarray(inputs["ninf_mask"], np.float32)

    in_maps = []
    for b in range(B):
        d1 = d1s.copy()
        d1[:, EJ0 : EJ0 + J] = ejs[b].T
        d1[:, EM0 : EM0 + M] = ems[b].T
        d2 = d2s.copy()
        d2[0:J, MK0 : MK0 + M] = msks[b]
        in_maps.append({
            "data1": d1.astype(ml_dtypes.bfloat16),
            "data2": d2.astype(ml_dtypes.bfloat16),
        })

    last_results = run_bass_kernel_spmd(_NC, in_maps, core_ids=list(range(B)))
    out = np.stack(
        [last_results.results[b]["out"].reshape(J * M) for b in range(B)]
    )
    return out.astype(np.float32)
